# revision 1
# baseline (speedup 1.0000x reference)
"""Trainium2 Bass kernel v2 for nn_DecoderLayer_31086973288870.

Full decoder layer (QKV -> causal attention -> out-proj -> LN -> FFN -> LN),
S=2048, D=2048, 16 heads, INNER=8192, batch 1.

Sharding (8 cores): heads tensor-parallel for attention (2 heads/core),
AllToAll (split per head, bf16) to seq-parallel for out-proj/LN/FFN with
streamed full weights.

v2 changes vs baseline:
  - bf16 everywhere on matmul inputs (xT, wqkv, v, pt, lin_w, oT/A2A, FFN);
    fp32 PSUM accumulation + fp32 LN/softmax statistics.
  - kt-outer accumulation during the xT stream (v st0-3 + q-head0 share the
    8 PSUM banks), so the PE works while x arrives.
  - attention h0 -> stage+A2A#1 overlapped with q/k head1 + attention h1;
    A2A#2 overlapped with out-proj evens-first accumulation.
  - causal diagonal slicing: score/AV/se matmuls and exp/mask shrink to the
    unmasked query range on diagonal tiles.
  - biases folded into DVE/ACT drains (no ones-row bias matmuls).
  - LN stats accumulated per 512-chunk as out-proj/FFN2 chunks land;
    finalize + normalize + (transpose|DMA-out) pipelined per chunk.
"""

import math
import sys

import numpy as np

try:
    import concourse.bass as bass  # noqa: F401
except ImportError:  # pragma: no cover
    sys.path.insert(0, "/opt/trn_rl_repo")
    import concourse.bass as bass  # noqa: F401

import ml_dtypes
import concourse.mybir as mybir
import concourse.tile as tile
from concourse import bacc
from concourse.bass_utils import run_bass_kernel_spmd
from concourse.masks import make_identity
from contextlib import ExitStack

S = 2048
D = 2048
HEADS = 16
HD = 128
INNER = 8192
NCORES = 8
HPC = HEADS // NCORES     # heads per core = 2
HDC = HPC * HD            # head dims per core = 256
SC = S // NCORES          # seq rows per core = 256
EPS = 1e-5
RSQ = 1.0 / math.sqrt(float(D))

f32 = mybir.dt.float32
FP = mybir.dt.float32r
bf16 = mybir.dt.bfloat16
AF = mybir.ActivationFunctionType
OP = mybir.AluOpType
AX = mybir.AxisListType


def _build(nocc=False, w1bufs=4, w2bufs=3, linbufs=3, ptbufs=6, spsbufs=2):
    nc = bacc.Bacc("TRN2", target_bir_lowering=False, debug=False,
                   num_devices=NCORES)

    def din(name, shape, dt):
        return nc.dram_tensor(name, shape, dt, kind="ExternalInput").ap()

    def dout(name, shape, dt):
        return nc.dram_tensor(name, shape, dt, kind="ExternalOutput").ap()

    xT_d = din("xT", [D, S], bf16)
    xs_d = din("x_slice", [SC, D], f32)
    wq_d = din("wq", [D, HDC], bf16)
    wk_d = din("wk", [D, HDC], bf16)
    wv_d = din("wv", [D, HDC], bf16)
    bq_d = din("bq", [HDC], f32)
    bk_d = din("bk", [HDC], f32)
    bv_d = din("bv", [HDC], f32)
    linw_d = din("lin_w", [D, D], bf16)
    linb_d = din("lin_b", [D], f32)
    ff1_d = din("ff1_w", [D, INNER], bf16)
    ff1b_d = din("ff1_b", [INNER], f32)
    ff2_d = din("ff2_w", [INNER, D], bf16)
    ff2b_d = din("ff2_b", [D], f32)
    ln1g_d = din("ln1_g", [D], f32)
    ln1b_d = din("ln1_b", [D], f32)
    ln2g_d = din("ln2_g", [D], f32)
    ln2b_d = din("ln2_b", [D], f32)
    out_d = dout("out_slice", [SC, D], f32)

    wdma = nc.scalar.dma_start      # weights ring (HWDGE via ACT queue)
    w2dma = nc.gpsimd.dma_start     # second weights ring (FFN2 stream)
    sdma = nc.sync.dma_start        # activations/staging ring

    with tile.TileContext(nc) as tc, ExitStack() as ctx:
        const = ctx.enter_context(tc.tile_pool(name="const", bufs=1))
        dram = ctx.enter_context(tc.tile_pool(name="dram", bufs=1, space="DRAM"))
        stat = ctx.enter_context(tc.tile_pool(name="stat", bufs=8))
        sq_scratch_pool = ctx.enter_context(
            tc.tile_pool(name="sqscr", bufs=3))

        ident_f = const.tile([128, 128], f32)
        make_identity(nc, ident_f[:])
        ident = const.tile([128, 128], FP)
        nc.vector.tensor_copy(ident[:], ident_f[:])
        eps_sb = const.tile([128, 1], f32)
        nc.gpsimd.memset(eps_sb[:], EPS)
        onesf = const.tile([128, 1], f32)
        nc.gpsimd.memset(onesf[:], 1.0)
        ones_col = const.tile([128, 1], bf16)
        nc.vector.tensor_copy(ones_col[:], onesf[:])
        # maskbig[i, u] = 1.0 iff u >= i + 384; slice [384-d : 896-d] keeps
        # key j (partition i) for query col u iff j <= u - delta.
        maskf = const.tile([128, 896], f32)
        nc.gpsimd.memset(maskf[:], 1.0)
        nc.gpsimd.affine_select(
            out=maskf[:], in_=maskf[:], compare_op=OP.is_ge, fill=0.0,
            base=-384, channel_multiplier=-1, pattern=[[1, 896]])
        maskbig = const.tile([128, 896], bf16)
        nc.vector.tensor_copy(maskbig[:], maskf[:])

        bq_sb = const.tile([128, HPC], f32)
        sdma(bq_sb[:], bq_d.rearrange("(h p) -> p h", p=128))
        bk_sb = const.tile([128, HPC], f32)
        sdma(bk_sb[:], bk_d.rearrange("(h p) -> p h", p=128))
        bv_row = const.tile([1, HDC], f32)
        sdma(bv_row[:], bv_d[None, :])
        bvbc = const.tile([128, HDC], f32)
        nc.gpsimd.partition_broadcast(bvbc[:], bv_row[:])
        ff1b_sb = const.tile([128, INNER // 128], f32)
        sdma(ff1b_sb[:], ff1b_d.rearrange("(t p) -> p t", p=128))

        def broadcast_row(pool, row_d, tag):
            t = pool.tile([128, D], f32, tag=tag, name=tag, bufs=1)
            sdma(t[0:1, :], row_d[None, :])
            nc.gpsimd.partition_broadcast(t[:], t[0:1, :])
            return t


        # ---------------- Phase 1: QKV projections -----------------------
        # Pool open order is the REVERSE of close order (LIFO): the
        # attention-lifetime pools (qk/v/oT/pt/sm/att_ps) open first, the
        # QKV-only pools (xT, wv, wqk) open last so they can close mid-way.
        qkv_keep = ExitStack()
        qk_pool = qkv_keep.enter_context(tc.tile_pool(name="qk", bufs=1))
        v_pool = qkv_keep.enter_context(tc.tile_pool(name="v", bufs=1))
        qT = {}
        kT = {}
        v_sb = []
        for h in range(HPC):
            qT[h] = qk_pool.tile([128, S], bf16, tag=f"qT{h}", name=f"qT{h}")
            kT[h] = qk_pool.tile([128, S], bf16, tag=f"kT{h}", name=f"kT{h}")
        for st in range(16):
            v_sb.append(v_pool.tile([128, HDC], bf16, tag=f"v{st}",
                                    name=f"v{st}"))

        att_keep = ExitStack()
        ot_pool = att_keep.enter_context(tc.tile_pool(name="oT", bufs=1))
        oT = [ot_pool.tile([128, S], bf16, tag=f"oT{h}", name=f"oT{h}")
              for h in range(HPC)]
        ptp = att_keep.enter_context(tc.tile_pool(name="pT", bufs=6))
        smp = att_keep.enter_context(tc.tile_pool(name="att_sm", bufs=2))
        app_keep = ExitStack()
        app = None  # PSUM pool opened after the QKV wave pools close

        xT_keep = ExitStack()
        xp = xT_keep.enter_context(tc.tile_pool(name="xT", bufs=1))
        xT_sb = []
        for i in range(16):
            t = xp.tile([128, S], bf16, tag=f"xT{i}", name=f"xT{i}")
            sdma(t[:], xT_d[i * 128:(i + 1) * 128, :])
            xT_sb.append(t)

        wvp = xT_keep.enter_context(tc.tile_pool(name="wv", bufs=1))
        wqkp = xT_keep.enter_context(tc.tile_pool(name="wqk", bufs=4))
        wv_sb = wvp.tile([128, 16, HDC], bf16, tag="wv", name="wv")
        wdma(wv_sb[:], wv_d.rearrange("(k p) c -> p k c", p=128))
        wq_t = {}
        wk_t = {}
        for h in range(HPC):
            wq_t[h] = wqkp.tile([128, 16, 128], bf16, tag=f"wq{h}",
                                name=f"wq{h}", bufs=1)
            wdma(wq_t[h][:],
                 wq_d[:, h * 128:(h + 1) * 128]
                 .rearrange("(k p) c -> p k c", p=128))
        for h in range(HPC):
            wk_t[h] = wqkp.tile([128, 16, 128], bf16, tag=f"wk{h}",
                                name=f"wk{h}", bufs=1)
            wdma(wk_t[h][:],
                 wk_d[:, h * 128:(h + 1) * 128]
                 .rearrange("(k p) c -> p k c", p=128))

        def qk_drain(dst, h, qs, ps, b_sb):
            nc.scalar.activation(
                dst[:, qs * 512:(qs + 1) * 512], ps[:],
                AF.Identity, bias=b_sb[:, h:h + 1])

        def v_drain(st, ps):
            nc.vector.tensor_tensor(v_sb[st][:], ps[:], bvbc[:], OP.add)

        # Wave 1 (kt-outer, paced by xT arrival): v st0-3 + q head0 qs0-3.
        with tc.tile_pool(name="w1ps", bufs=8, space="PSUM") as pp:
            vps = [pp.tile([128, HDC], f32, tag=f"w1v{g}", name=f"w1v{g}",
                           bufs=1) for g in range(4)]
            qps = [pp.tile([128, 512], f32, tag=f"w1q{g}", name=f"w1q{g}",
                           bufs=1) for g in range(4)]
            for kt in range(16):
                for g in range(4):
                    nc.tensor.matmul(
                        vps[g][:], xT_sb[kt][:, g * 128:(g + 1) * 128],
                        wv_sb[:, kt, :], start=(kt == 0), stop=(kt == 15))
                for g in range(4):
                    nc.tensor.matmul(
                        qps[g][:], wq_t[0][:, kt, :],
                        xT_sb[kt][:, g * 512:(g + 1) * 512],
                        start=(kt == 0), stop=(kt == 15))
            for g in range(4):
                v_drain(g, vps[g])
            for g in range(4):
                qk_drain(qT[0], 0, g, qps[g], bq_sb)

        # Wave 2: v st4-15 (dense, xT resident), then k head0.
        with tc.tile_pool(name="w2ps", bufs=4, space="PSUM") as pp:
            for st in range(4, 16):
                ps = pp.tile([128, HDC], f32, tag="w2v", name="w2v")
                for kt in range(16):
                    nc.tensor.matmul(
                        ps[:], xT_sb[kt][:, st * 128:(st + 1) * 128],
                        wv_sb[:, kt, :], start=(kt == 0), stop=(kt == 15))
                v_drain(st, ps)
            for qs in range(4):
                ps = pp.tile([128, 512], f32, tag="w2k", name="w2k")
                for kt in range(16):
                    nc.tensor.matmul(
                        ps[:], wk_t[0][:, kt, :],
                        xT_sb[kt][:, qs * 512:(qs + 1) * 512],
                        start=(kt == 0), stop=(kt == 15))
                qk_drain(kT[0], 0, qs, ps, bk_sb)
        app = app_keep.enter_context(
            tc.tile_pool(name="att_ps", bufs=1, space="PSUM"))

        # ---------------- Phase 2: attention + split A2A ------------------
        a2a_in = dram.tile([NCORES, HDC, SC], bf16, tag="a2ai",
                           name="a2ai")
        a2a_out = dram.tile([NCORES, HDC, SC], bf16, tag="a2ao",
                            name="a2ao")

        def attention_head(h):
            for qs in range(4):
                kmax = 4 * qs + 4
                o_ps = app.tile([128, 512], f32, tag="o_ps", name="o_ps",
                                bufs=2)
                se_ps = app.tile([1, 512], f32, tag="se_ps", name="se_ps",
                                 bufs=2)
                for kt in range(kmax):
                    delta = kt * 128 - qs * 512
                    lo = max(delta, 0)          # first unmasked query col
                    s_ps = app.tile([128, 512], f32, tag="s_ps",
                                    name="s_ps", bufs=spsbufs)
                    nc.tensor.matmul(
                        s_ps[:, lo:512], kT[h][:, kt * 128:(kt + 1) * 128],
                        qT[h][:, qs * 512 + lo:(qs + 1) * 512],
                        start=True, stop=True)
                    pt = ptp.tile([128, 512], bf16, tag="pt", name="pt")
                    nc.scalar.activation(pt[:, lo:512], s_ps[:, lo:512],
                                         AF.Exp, scale=RSQ)
                    if delta >= 0:
                        nc.vector.tensor_tensor(
                            pt[:, lo:512], pt[:, lo:512],
                            maskbig[:, 384 - delta + lo:896 - delta],
                            OP.mult)
                    nc.tensor.matmul(
                        o_ps[:, lo:512], v_sb[kt][:, h * 128:(h + 1) * 128],
                        pt[:, lo:512], start=(kt == 0),
                        stop=(kt == kmax - 1))
                    nc.tensor.matmul(
                        se_ps[:, lo:512], ones_col[:], pt[:, lo:512],
                        start=(kt == 0), stop=(kt == kmax - 1))
                se_sb = smp.tile([1, 512], f32, tag="se_sb", name="se_sb")
                nc.vector.tensor_copy(se_sb[:], se_ps[:])
                rec = smp.tile([1, 512], f32, tag="rec", name="rec")
                nc.vector.reciprocal(rec[:], se_sb[:])
                bc = smp.tile([128, 512], f32, tag="bc", name="bc")
                nc.gpsimd.partition_broadcast(bc[:], rec[:])
                nc.vector.tensor_tensor(
                    oT[h][:, qs * 512:(qs + 1) * 512], o_ps[:], bc[:],
                    OP.mult)

        def stage_head(h):
            for c in range(NCORES):
                sdma(a2a_in[c, h * 128:(h + 1) * 128, :],
                     oT[h][:, c * SC:(c + 1) * SC])

        def run_a2a():
            if nocc:
                sdma(a2a_out[:], a2a_in[:])
            else:
                nc.gpsimd.collective_compute(
                    "AllToAll", OP.bypass,
                    replica_groups=[list(range(NCORES))],
                    ins=[a2a_in[:]], outs=[a2a_out[:]])

        attention_head(0)
        stage_head(0)

        # q/k head1 (overlaps A2A#1), then attention head1.
        with tc.tile_pool(name="w3ps", bufs=2, space="PSUM") as pp:
            for (w_t, b_sb, dst) in ((wq_t[1], bq_sb, qT[1]),
                                     (wk_t[1], bk_sb, kT[1])):
                for qs in range(4):
                    ps = pp.tile([128, 512], f32, tag="w3qk", name="w3qk")
                    for kt in range(16):
                        nc.tensor.matmul(
                            ps[:], w_t[:, kt, :],
                            xT_sb[kt][:, qs * 512:(qs + 1) * 512],
                            start=(kt == 0), stop=(kt == 15))
                    qk_drain(dst, 1, qs, ps, b_sb)
        xT_keep.close()

        attention_head(1)
        stage_head(1)
        run_a2a()
        app_keep.close()
        att_keep.close()
        qkv_keep.close()

        # ---------------- Phase 3: out-proj + LN1 (chunked stats) --------
        resA = ExitStack()
        res_pool = resA.enter_context(tc.tile_pool(name="res", bufs=1))
        h1b = [res_pool.tile([128, D], f32, tag=f"h1b{m}", name=f"h1b{m}")
               for m in range(2)]
        u2 = [res_pool.tile([128, D], f32, tag=f"u2{m}", name=f"u2{m}")
              for m in range(2)]
        h1T_keep = ExitStack()
        h1Tp = h1T_keep.enter_context(tc.tile_pool(name="h1T", bufs=1))
        h1T = [h1Tp.tile([128, SC], bf16, tag=f"h1T{kt}", name=f"h1T{kt}")
               for kt in range(16)]

        up_keep = ExitStack()
        up = up_keep.enter_context(tc.tile_pool(name="up", bufs=1))
        u_tiles = [up.tile([128, D], f32, tag=f"u{m}", name=f"u{m}")
                   for m in range(2)]
        h1 = [up.tile([128, D], FP, tag=f"h1_{m}", name=f"h1_{m}")
              for m in range(2)]
        lnA_keep = ExitStack()
        lnA = lnA_keep.enter_context(tc.tile_pool(name="lnA", bufs=1))
        LINB = broadcast_row(lnA, linb_d, "LINB")
        G1 = broadcast_row(lnA, ln1g_d, "G1")
        B1 = broadcast_row(lnA, ln1b_d, "B1")
        B2f = broadcast_row(lnA, ff2b_d, "B2f")

        xsb = []
        for m in range(2):
            t = up.tile([128, D], f32, tag=f"xs{m}", name=f"xs{m}")
            sdma(t[:], xs_d[m * 128:(m + 1) * 128, :])
            xsb.append(t)
        for m in range(2):
            nc.vector.tensor_tensor(xsb[m][:], xsb[m][:], LINB[:], OP.add)

        # chunked LN state: per (m, n) partial sums
        ln1_mu = [[stat.tile([128, 1], f32, tag=f"l1mu{m}_{n}",
                             name=f"l1mu{m}_{n}", bufs=1)
                   for n in range(4)] for m in range(2)]
        ln1_sq = [[stat.tile([128, 1], f32, tag=f"l1sq{m}_{n}",
                             name=f"l1sq{m}_{n}", bufs=1)
                   for n in range(4)] for m in range(2)]
        ln2_mu = [[stat.tile([128, 1], f32, tag=f"l2mu{m}_{n}",
                             name=f"l2mu{m}_{n}", bufs=1)
                   for n in range(4)] for m in range(2)]
        ln2_sq = [[stat.tile([128, 1], f32, tag=f"l2sq{m}_{n}",
                             name=f"l2sq{m}_{n}", bufs=1)
                   for n in range(4)] for m in range(2)]


        def chunk_stats(u_tile, n, mu_t, sq_t):
            ch = u_tile[:, n * 512:(n + 1) * 512]
            nc.vector.reduce_sum(mu_t[:], ch, axis=AX.X)
            scr = sq_scratch_pool.tile([128, 512], f32, tag="sqs",
                                       name="sqs")
            nc.scalar.activation(scr[:], ch, AF.Square, accum_out=sq_t[:])

        def ln_finalize(mu_list, sq_list, tag):
            musum = stat.tile([128, 1], f32, tag=f"msum{tag}",
                              name=f"msum{tag}", bufs=2)
            nc.vector.tensor_tensor(musum[:], mu_list[0][:], mu_list[1][:],
                                    OP.add)
            nc.vector.tensor_tensor(musum[:], musum[:], mu_list[2][:],
                                    OP.add)
            nc.vector.tensor_tensor(musum[:], musum[:], mu_list[3][:],
                                    OP.add)
            sqsum = stat.tile([128, 1], f32, tag=f"ssum{tag}",
                              name=f"ssum{tag}", bufs=2)
            nc.vector.tensor_tensor(sqsum[:], sq_list[0][:], sq_list[1][:],
                                    OP.add)
            nc.vector.tensor_tensor(sqsum[:], sqsum[:], sq_list[2][:],
                                    OP.add)
            nc.vector.tensor_tensor(sqsum[:], sqsum[:], sq_list[3][:],
                                    OP.add)
            mu = stat.tile([128, 1], f32, tag=f"mu{tag}", name=f"mu{tag}",
                           bufs=2)
            nc.vector.tensor_scalar(mu[:], musum[:], 1.0 / D, None, OP.mult)
            ex2 = stat.tile([128, 1], f32, tag=f"ex2{tag}",
                            name=f"ex2{tag}", bufs=2)
            nc.vector.tensor_scalar(ex2[:], sqsum[:], 1.0 / D, None, OP.mult)
            mu2 = stat.tile([128, 1], f32, tag=f"mu2{tag}", name=f"mu2{tag}",
                            bufs=2)
            nc.vector.tensor_tensor(mu2[:], mu[:], mu[:], OP.mult)
            var = stat.tile([128, 1], f32, tag=f"var{tag}", name=f"var{tag}",
                            bufs=2)
            nc.vector.tensor_tensor(var[:], ex2[:], mu2[:], OP.subtract)
            std = stat.tile([128, 1], f32, tag=f"std{tag}", name=f"std{tag}",
                            bufs=2)
            nc.scalar.activation(std[:], var[:], AF.Sqrt, bias=eps_sb[:])
            rstd = stat.tile([128, 1], f32, tag=f"rstd{tag}",
                             name=f"rstd{tag}", bufs=2)
            nc.vector.reciprocal(rstd[:], std[:])
            return mu, rstd

        def ln_norm_chunk(dst, src_tile, n, mu, rstd, G, B):
            ch = src_tile[:, n * 512:(n + 1) * 512]
            o = dst[:, n * 512:(n + 1) * 512]
            nc.vector.tensor_scalar(o, ch, mu[:], rstd[:],
                                    OP.subtract, OP.mult)
            nc.vector.tensor_tensor(o, o, G[:, n * 512:(n + 1) * 512],
                                    OP.mult)
            nc.vector.tensor_tensor(o, o, B[:, n * 512:(n + 1) * 512],
                                    OP.add)

        with tc.tile_pool(name="linw", bufs=linbufs) as lwp, \
             tc.tile_pool(name="ofT", bufs=1) as ofp, \
             tc.tile_pool(name="op_ps", bufs=4, space="PSUM") as opp:
            lwt = []
            for n in range(4):
                t = lwp.tile([128, 16, 512], bf16, tag="lw", name=f"lw{n}")
                wdma(t[:],
                     linw_d[:, n * 512:(n + 1) * 512]
                     .rearrange("(k p) c -> p k c", p=128))
                lwt.append(t)
            ofT = [[None] * NCORES for _ in range(HPC)]
            for h in range(HPC):
                for c in range(NCORES):
                    t = ofp.tile([128, SC], bf16, tag=f"ofT{h}_{c}",
                                 name=f"ofT{h}_{c}")
                    sdma(t[:], a2a_out[c, h * 128:(h + 1) * 128, :])
                    ofT[h][c] = t
            for n in range(4):
                for m in range(2):
                    ps = opp.tile([128, 512], f32, tag=f"op_ps{m}",
                                  name=f"op_ps{m}")
                    first = True
                    for h in range(HPC):        # evens (h=0) first
                        for c in range(NCORES):
                            kt = 2 * c + h
                            nc.tensor.matmul(
                                ps[:], ofT[h][c][:, m * 128:(m + 1) * 128],
                                lwt[n][:, kt, :],
                                start=first,
                                stop=(h == HPC - 1 and c == NCORES - 1))
                            first = False
                    nc.vector.tensor_tensor(
                        u_tiles[m][:, n * 512:(n + 1) * 512], ps[:],
                        xsb[m][:, n * 512:(n + 1) * 512], OP.add)
                    chunk_stats(u_tiles[m], n, ln1_mu[m][n], ln1_sq[m][n])

        # ---------------- Phase 4: LN1 finalize + transpose --------------
        with tc.tile_pool(name="tr_ps", bufs=3, space="PSUM") as tpp:
            mr = [ln_finalize(ln1_mu[m], ln1_sq[m], f"1_{m}")
                  for m in range(2)]
            for n in range(4):
                for m in range(2):
                    ln_norm_chunk(h1[m][:], u_tiles[m], n, mr[m][0],
                                  mr[m][1], G1, B1)
                for kt in range(4 * n, 4 * n + 4):
                    for m in range(2):
                        tp = tpp.tile([128, 128], FP, tag="tr_ps",
                                      name="tr_ps")
                        nc.tensor.transpose(
                            tp[:], h1[m][:, kt * 128:(kt + 1) * 128],
                            ident[:])
                        nc.vector.tensor_copy(
                            h1T[kt][:, m * 128:(m + 1) * 128], tp[:])
            for m in range(2):
                nc.vector.tensor_tensor(h1b[m][:], h1[m][:], B2f[:], OP.add)
        lnA_keep.close()
        up_keep.close()

        # ---------------- Phase 5: FFN1 (seq-parallel) -------------------
        gi_keep = ExitStack()
        gip = gi_keep.enter_context(tc.tile_pool(name="gi", bufs=1))
        w2_keep = ExitStack()
        w2p = w2_keep.enter_context(tc.tile_pool(name="w2", bufs=w2bufs))
        with tc.tile_pool(name="w1", bufs=w1bufs) as w1p, \
             tc.tile_pool(name="ffn_ps", bufs=3, space="PSUM") as fpp:
            ginner = []
            for ib in range(16):
                w1t = w1p.tile([128, 16, 512], bf16, tag="w1", name="w1t")
                wdma(w1t[:],
                     ff1_d[:, ib * 512:(ib + 1) * 512]
                     .rearrange("(k p) c -> p k c", p=128))
                for ms in range(4):
                    it = ib * 4 + ms
                    ps = fpp.tile([128, SC], f32, tag="f1_ps", name="f1_ps")
                    for kt in range(16):
                        nc.tensor.matmul(
                            ps[:], w1t[:, kt, ms * 128:(ms + 1) * 128],
                            h1T[kt][:], start=(kt == 0), stop=(kt == 15))
                    g = gip.tile([128, SC], bf16, tag=f"gi{it}",
                                 name=f"gi{it}")
                    nc.scalar.activation(g[:], ps[:], AF.Gelu,
                                         bias=ff1b_sb[:, it:it + 1])
                    ginner.append(g)

        # -------- Phase 6: FFN2 + LN2 chunked stats ----------------------
        with tc.tile_pool(name="lnB", bufs=1) as lnB, \
             tc.tile_pool(name="f2_ps", bufs=4, space="PSUM") as f2p:
            G2 = broadcast_row(lnB, ln2g_d, "G2")
            B2 = broadcast_row(lnB, ln2b_d, "B2")
            for n in range(4):
                pss = [f2p.tile([128, 512], f32, tag=f"f2_ps{m}",
                                name=f"f2ps{m}") for m in range(2)]
                for ktc in range(4):
                    w2t = w2p.tile([128, 16, 512], bf16, tag="w2", name="w2t")
                    w2dma(w2t[:],
                         ff2_d[ktc * 2048:(ktc + 1) * 2048,
                               n * 512:(n + 1) * 512]
                         .rearrange("(k p) c -> p k c", p=128))
                    for m in range(2):
                        for k2 in range(16):
                            kt = ktc * 16 + k2
                            nc.tensor.matmul(
                                pss[m][:],
                                ginner[kt][:, m * 128:(m + 1) * 128],
                                w2t[:, k2, :],
                                start=(kt == 0), stop=(kt == 63))
                for m in range(2):
                    nc.vector.tensor_tensor(
                        u2[m][:, n * 512:(n + 1) * 512], pss[m][:],
                        h1b[m][:, n * 512:(n + 1) * 512], OP.add)
                    chunk_stats(u2[m], n, ln2_mu[m][n], ln2_sq[m][n])

            # -------- Phase 7: LN2 finalize + store ----------------------
            for m in range(2):
                mu, rstd = ln_finalize(ln2_mu[m], ln2_sq[m], f"2_{m}")
                for n in range(4):
                    ln_norm_chunk(u2[m][:], u2[m], n, mu, rstd, G2, B2)
                    sdma(out_d[m * 128:(m + 1) * 128,
                               n * 512:(n + 1) * 512],
                         u2[m][:, n * 512:(n + 1) * 512])
        w2_keep.close()
        gi_keep.close()
        h1T_keep.close()
        resA.close()

    nc.compile()
    return nc


_NC_CACHE = {}


def _get_nc(debug=False, nocc=False, **kw):
    key = (nocc, tuple(sorted(kw.items())))
    if key not in _NC_CACHE:
        _NC_CACHE[key] = _build(nocc, **kw)
    return _NC_CACHE[key]


def make_in_maps(x, C_w, C_b, lin_w, lin_b, ff1_w, ff1_b, ff2_w, ff2_b,
                 ln1_g, ln1_b, ln2_g, ln2_b):
    x2 = np.asarray(x, dtype=np.float32)[0]            # [S, D]
    xT = np.ascontiguousarray(x2.T).astype(ml_dtypes.bfloat16)
    C_w = np.asarray(C_w, dtype=np.float32)
    C_b = np.asarray(C_b, dtype=np.float32)
    common = {
        "xT": xT,
        "lin_w": np.asarray(lin_w).astype(ml_dtypes.bfloat16),
        "lin_b": np.asarray(lin_b, dtype=np.float32),
        "ff1_w": np.asarray(ff1_w).astype(ml_dtypes.bfloat16),
        "ff1_b": np.asarray(ff1_b, dtype=np.float32),
        "ff2_w": np.asarray(ff2_w).astype(ml_dtypes.bfloat16),
        "ff2_b": np.asarray(ff2_b, dtype=np.float32),
        "ln1_g": np.asarray(ln1_g, dtype=np.float32),
        "ln1_b": np.asarray(ln1_b, dtype=np.float32),
        "ln2_g": np.asarray(ln2_g, dtype=np.float32),
        "ln2_b": np.asarray(ln2_b, dtype=np.float32),
    }
    in_maps = []
    for c in range(NCORES):
        sl = slice(c * HDC, (c + 1) * HDC)
        m = dict(common)
        m["wq"] = np.ascontiguousarray(C_w[:, sl]).astype(ml_dtypes.bfloat16)
        m["wk"] = np.ascontiguousarray(
            C_w[:, D:][:, sl]).astype(ml_dtypes.bfloat16)
        m["wv"] = np.ascontiguousarray(
            C_w[:, 2 * D:][:, sl]).astype(ml_dtypes.bfloat16)
        m["bq"] = np.ascontiguousarray(C_b[sl])
        m["bk"] = np.ascontiguousarray(C_b[D:][sl])
        m["bv"] = np.ascontiguousarray(C_b[2 * D:][sl])
        m["x_slice"] = np.ascontiguousarray(x2[c * SC:(c + 1) * SC, :])
        in_maps.append(m)
    return in_maps


def run(in_maps, debug=False):
    nc = _get_nc(debug)
    return run_bass_kernel_spmd(nc, in_maps, list(range(NCORES)))


def kernel(**inputs):
    in_maps = make_in_maps(**inputs)
    res = run(in_maps)
    out = np.concatenate(
        [res.results[c]["out_slice"] for c in range(NCORES)], axis=0)
    return out.reshape(1, S, D).astype(np.float32)



# revision 27
# speedup vs baseline: 2.8471x; 2.8471x over previous
"""Trainium2 Bass kernel v3 for nn_DecoderLayer_31086973288870.

Full decoder layer (QKV -> causal attention -> out-proj -> LN -> FFN -> LN),
S=2048, D=2048, 16 heads, INNER=8192, batch 1.

Sharding (8 cores): heads tensor-parallel for attention (2 heads/core),
per-head AllToAll (bf16) to seq-parallel for out-proj/LN/FFN with
streamed full weights.

v3 changes vs v2:
  - A2A split per head: A2A#0 overlaps qk-head1 + attention head1;
    A2A#1 overlaps the out-proj evens pass (all 8 PSUM banks).
  - attention kt loop software-pipelined depth 2 (score kt+2 issued
    before AV kt) so PE matmuls stay back-to-back past the ACT exp.
  - ln1_g folded into ff1_w host-side; b1' = ff1_b + ln1_b@ff1_w; FFN1
    consumes the normalized z directly.  Residual h1b = z32*G1 +
    (ln1_b+ff2_b) on DVE, off the critical path.
  - ln2 gamma/beta applied on host after gather; lin_b folded into the
    x_slice on host.
  - queue-mode pool allocator (out-of-order pool release) so weight
    streams (lin_w early; ff1 on sync ring; ff2 on gpsimd ring) overlap
    attention/out-proj without SBUF watermark blowup.
  - softmax denominators via reciprocal_approx_fast.
"""

import math
import sys

import numpy as np

try:
    import concourse.bass as bass  # noqa: F401
except ImportError:  # pragma: no cover
    sys.path.insert(0, "/opt/trn_rl_repo")
    import concourse.bass as bass  # noqa: F401

import ml_dtypes
import concourse.mybir as mybir
import concourse.tile as tile
from concourse import bacc
from concourse.bass_utils import run_bass_kernel_spmd
from concourse.masks import make_identity
from contextlib import ExitStack

S = 2048
D = 2048
HEADS = 16
HD = 128
INNER = 8192
NCORES = 8
HPC = HEADS // NCORES     # heads per core = 2
HDC = HPC * HD            # head dims per core = 256
SC = S // NCORES          # seq rows per core = 256
EPS = 1e-5
RSQ = 1.0 / math.sqrt(float(D))

f32 = mybir.dt.float32
FP = mybir.dt.float32r
bf16 = mybir.dt.bfloat16
AF = mybir.ActivationFunctionType
OP = mybir.AluOpType
AX = mybir.AxisListType


def _build(nocc=False, w1bufs=4, w2bufs=3, ptbufs=3, spsbufs=3):
    nc = bacc.Bacc("TRN2", target_bir_lowering=False, debug=False,
                   num_devices=NCORES)

    def din(name, shape, dt):
        return nc.dram_tensor(name, shape, dt, kind="ExternalInput").ap()

    def dout(name, shape, dt):
        return nc.dram_tensor(name, shape, dt, kind="ExternalOutput").ap()

    xT_d = din("xT", [D, S], bf16)
    xs_d = din("x_slice", [SC, D], f32)       # pre-biased with lin_b (host)
    wq_d = din("wq", [D, HDC], bf16)
    wk_d = din("wk", [D, HDC], bf16)
    wv_d = din("wv", [D, HDC], bf16)
    bq_d = din("bq", [HDC], f32)
    bk_d = din("bk", [HDC], f32)
    bv_d = din("bv", [HDC], f32)
    linw_d = din("lin_w", [D, D], bf16)
    ff1_d = din("ff1_w", [D, INNER], bf16)    # pre-scaled by ln1_g (host)
    ff1b_d = din("ff1_b", [INNER], f32)       # ff1_b + ln1_b @ ff1_w (host)
    ff2_d = din("ff2_w", [INNER, D], bf16)
    g1_d = din("g1", [D], f32)                # ln1_g
    bcomb_d = din("bcomb", [D], f32)          # ln1_b + ff2_b
    out_d = dout("out_slice", [SC, D], f32)   # z2 (pre gamma2/beta2)

    wdma = nc.scalar.dma_start      # ACT HWDGE: qkv weights + lin_w
    w2dma = nc.gpsimd.dma_start     # SWDGE: ff2 stream
    sdma = nc.sync.dma_start        # SP HWDGE: acts/staging + ff1 stream

    with tile.TileContext(nc, pool_alloc_mode="queue") as tc, \
         ExitStack() as ctx:
        const = ctx.enter_context(tc.tile_pool(name="const", bufs=1))
        dram = ctx.enter_context(tc.tile_pool(name="dram", bufs=1,
                                              space="DRAM"))
        stat = ctx.enter_context(tc.tile_pool(name="stat", bufs=8))
        sq_scratch_pool = None  # opened at phase 3 (out-proj)

        ident = const.tile([128, 128], FP)
        eps_sb = const.tile([128, 1], f32)
        nc.gpsimd.memset(eps_sb[:], EPS)
        zc = const.tile([128, 1], f32)
        nc.gpsimd.memset(zc[:], 0.0)
        onesf = const.tile([128, 1], f32)
        nc.gpsimd.memset(onesf[:], 1.0)
        ones_col = const.tile([128, 1], bf16)
        nc.vector.tensor_copy(ones_col[:], onesf[:])
        # maskbig[i, u] = 1.0 iff u >= i + 384; slice [384-d : 896-d] keeps
        # key j (partition i) for query col u iff j <= u - delta.
        maskbig = const.tile([128, 896], bf16)
        with tc.tile_pool(name="cscratch", bufs=1) as csp:
            ident_f = csp.tile([128, 128], f32)
            make_identity(nc, ident_f[:])
            nc.vector.tensor_copy(ident[:], ident_f[:])
            maskf = csp.tile([128, 896], f32)
            nc.gpsimd.memset(maskf[:], 1.0)
            nc.gpsimd.affine_select(
                out=maskf[:], in_=maskf[:], compare_op=OP.is_ge, fill=0.0,
                base=-384, channel_multiplier=-1, pattern=[[1, 896]])
            nc.vector.tensor_copy(maskbig[:], maskf[:])

        bq_sb = const.tile([128, HPC], f32)
        sdma(bq_sb[:], bq_d.rearrange("(h p) -> p h", p=128))
        bk_sb = const.tile([128, HPC], f32)
        sdma(bk_sb[:], bk_d.rearrange("(h p) -> p h", p=128))
        bv_row = const.tile([1, HDC], f32)
        sdma(bv_row[:], bv_d[None, :])
        bvbc = const.tile([128, HDC], f32)
        nc.gpsimd.partition_broadcast(bvbc[:], bv_row[:])
        ff1b_sb = const.tile([128, INNER // 128], f32)
        sdma(ff1b_sb[:], ff1b_d.rearrange("(t p) -> p t", p=128))

        # ---------------- Phase 1: QKV projections -----------------------
        qkv_keep = ExitStack()
        qk_pool = qkv_keep.enter_context(tc.tile_pool(name="qk", bufs=1))
        v_pool = qkv_keep.enter_context(tc.tile_pool(name="v", bufs=1))
        qT = {}
        kT = {}
        v_sb = []
        for h in range(HPC):
            qT[h] = qk_pool.tile([128, S], bf16, tag=f"qT{h}", name=f"qT{h}")
            kT[h] = qk_pool.tile([128, S], bf16, tag=f"kT{h}", name=f"kT{h}")
        for st in range(16):
            v_sb.append(v_pool.tile([128, HDC], bf16, tag=f"v{st}",
                                    name=f"v{st}"))

        att_keep = ExitStack()
        ot_pool = att_keep.enter_context(tc.tile_pool(name="oT", bufs=1))
        oT = [ot_pool.tile([128, S], bf16, tag=f"oT{h}", name=f"oT{h}")
              for h in range(HPC)]
        ptp = att_keep.enter_context(tc.tile_pool(name="pT", bufs=ptbufs))
        smp = att_keep.enter_context(tc.tile_pool(name="att_sm", bufs=2))

        xT_keep = ExitStack()
        xp = xT_keep.enter_context(tc.tile_pool(name="xT", bufs=1))
        xT_sb = []
        for i in range(16):
            t = xp.tile([128, S], bf16, tag=f"xT{i}", name=f"xT{i}")
            sdma(t[:], xT_d[i * 128:(i + 1) * 128, :])
            xT_sb.append(t)

        wvp = xT_keep.enter_context(tc.tile_pool(name="wv", bufs=1))
        wqkp = xT_keep.enter_context(tc.tile_pool(name="wqk", bufs=4))
        wv_sb = wvp.tile([128, 16, HDC], bf16, tag="wv", name="wv")
        wdma(wv_sb[:], wv_d.rearrange("(k p) c -> p k c", p=128))
        wq_t = {}
        wk_t = {}
        for h in range(HPC):
            wq_t[h] = wqkp.tile([128, 16, 128], bf16, tag=f"wq{h}",
                                name=f"wq{h}", bufs=1)
            wdma(wq_t[h][:],
                 wq_d[:, h * 128:(h + 1) * 128]
                 .rearrange("(k p) c -> p k c", p=128))
        for h in range(HPC):
            wk_t[h] = wqkp.tile([128, 16, 128], bf16, tag=f"wk{h}",
                                name=f"wk{h}", bufs=1)
            wdma(wk_t[h][:],
                 wk_d[:, h * 128:(h + 1) * 128]
                 .rearrange("(k p) c -> p k c", p=128))

        # Shared weight-stream pool: 4 rotating [128,16,512]bf16 slots.
        # First holds the 4 lin_w tiles (prefetched on the ACT ring before
        # any ACT compute — transfers run during the QKV waves), then the
        # 16 ff1 tiles rotate through the same slots (each w1 load starts
        # as soon as the corresponding lin_w slot's last out-proj use is
        # done).  Right-side stack: closes after FFN1, out of order wrt
        # the left stack.
        wbig_keep = ExitStack()
        wbp = wbig_keep.enter_context(
            tc.tile_pool(name="wbig", bufs=4, side="right"))
        lwt = []
        for n in range(4):
            t = wbp.tile([128, 16, 512], bf16, tag="wbig", name=f"lw{n}")
            wdma(t[:],
                 linw_d[:, n * 512:(n + 1) * 512]
                 .rearrange("(k p) c -> p k c", p=128))
            lwt.append(t)

        def qk_drain(dst, h, qs, ps, b_sb):
            nc.scalar.activation(
                dst[:, qs * 512:(qs + 1) * 512], ps[:],
                AF.Identity, bias=b_sb[:, h:h + 1])

        def v_drain(st, ps):
            nc.vector.tensor_tensor(v_sb[st][:], ps[:], bvbc[:], OP.add)

        # Wave 1 (kt-outer, paced by xT arrival): v st0-3 + q head0 qs0-3.
        with tc.tile_pool(name="w1ps", bufs=8, space="PSUM") as pp:
            vps = [pp.tile([128, HDC], f32, tag=f"w1v{g}", name=f"w1v{g}",
                           bufs=1) for g in range(4)]
            qps = [pp.tile([128, 512], f32, tag=f"w1q{g}", name=f"w1q{g}",
                           bufs=1) for g in range(4)]
            for kt in range(16):
                for g in range(4):
                    nc.tensor.matmul(
                        vps[g][:], xT_sb[kt][:, g * 128:(g + 1) * 128],
                        wv_sb[:, kt, :], start=(kt == 0), stop=(kt == 15))
                for g in range(4):
                    nc.tensor.matmul(
                        qps[g][:], wq_t[0][:, kt, :],
                        xT_sb[kt][:, g * 512:(g + 1) * 512],
                        start=(kt == 0), stop=(kt == 15))
            for g in range(4):
                v_drain(g, vps[g])
            for g in range(4):
                qk_drain(qT[0], 0, g, qps[g], bq_sb)

        # Wave 2: k head0 first (attention h0 needs it), then v st4-15.
        with tc.tile_pool(name="w2ps", bufs=4, space="PSUM") as pp:
            for qs in range(4):
                ps = pp.tile([128, 512], f32, tag="w2k", name="w2k")
                for kt in range(16):
                    nc.tensor.matmul(
                        ps[:], wk_t[0][:, kt, :],
                        xT_sb[kt][:, qs * 512:(qs + 1) * 512],
                        start=(kt == 0), stop=(kt == 15))
                qk_drain(kT[0], 0, qs, ps, bk_sb)
            for st in range(4, 16):
                ps = pp.tile([128, HDC], f32, tag="w2v", name="w2v")
                for kt in range(16):
                    nc.tensor.matmul(
                        ps[:], xT_sb[kt][:, st * 128:(st + 1) * 128],
                        wv_sb[:, kt, :], start=(kt == 0), stop=(kt == 15))
                v_drain(st, ps)

        # ---------------- Phase 2: attention + split A2A ------------------
        a2a_in = [dram.tile([NCORES, HD, SC], bf16, tag=f"a2ai{h}",
                            name=f"a2ai{h}") for h in range(HPC)]
        a2a_out = [dram.tile([NCORES, HD, SC], bf16, tag=f"a2ao{h}",
                             name=f"a2ao{h}") for h in range(HPC)]

        def attention_head(h):
            with tc.tile_pool(name=f"att_ps{h}", bufs=1,
                              space="PSUM") as app:
                for qs in range(4):
                    kmax = 4 * qs + 4
                    o_ps = app.tile([128, 512], f32, tag="o_ps",
                                    name="o_ps", bufs=2)
                    se_ps = app.tile([1, 512], f32, tag="se_ps",
                                     name="se_ps", bufs=2)
                    sps = [None] * kmax
                    pts = [None] * kmax

                    def lo_of(kt):
                        return max(kt * 128 - qs * 512, 0)

                    def issue_score(kt):
                        lo = lo_of(kt)
                        s_ps = app.tile([128, 512], f32, tag="s_ps",
                                        name="s_ps", bufs=spsbufs)
                        nc.tensor.matmul(
                            s_ps[:, lo:512],
                            kT[h][:, kt * 128:(kt + 1) * 128],
                            qT[h][:, qs * 512 + lo:(qs + 1) * 512],
                            start=True, stop=True)
                        sps[kt] = s_ps

                    def issue_exp(kt):
                        delta = kt * 128 - qs * 512
                        lo = max(delta, 0)
                        pt = ptp.tile([128, 512], bf16, tag="pt", name="pt")
                        nc.scalar.activation(
                            pt[:, lo:512], sps[kt][:, lo:512],
                            AF.Exp, bias=zc[:], scale=RSQ)
                        if delta >= 0:
                            nc.vector.tensor_tensor(
                                pt[:, lo:512], pt[:, lo:512],
                                maskbig[:, 384 - delta + lo:896 - delta],
                                OP.mult)
                        pts[kt] = pt

                    def issue_av(kt):
                        lo = lo_of(kt)
                        nc.tensor.matmul(
                            o_ps[:, lo:512],
                            v_sb[kt][:, h * 128:(h + 1) * 128],
                            pts[kt][:, lo:512], start=(kt == 0),
                            stop=(kt == kmax - 1))
                        nc.tensor.matmul(
                            se_ps[:, lo:512], ones_col[:],
                            pts[kt][:, lo:512],
                            start=(kt == 0), stop=(kt == kmax - 1))

                    issue_score(0)
                    issue_score(1)
                    issue_exp(0)
                    for kt in range(kmax):
                        if kt + 2 < kmax:
                            issue_score(kt + 2)
                        if kt + 1 < kmax:
                            issue_exp(kt + 1)
                        issue_av(kt)

                    rec = smp.tile([1, 512], f32, tag="rec", name="rec")
                    nc.vector.reciprocal_approx_fast(rec[:], se_ps[:])
                    bc = smp.tile([128, 512], f32, tag="bc", name="bc")
                    nc.gpsimd.partition_broadcast(bc[:], rec[:])
                    nc.vector.tensor_tensor(
                        oT[h][:, qs * 512:(qs + 1) * 512], o_ps[:], bc[:],
                        OP.mult)
                    # stage this qs's two destination-core slices now
                    for c in (2 * qs, 2 * qs + 1):
                        sdma(a2a_in[h][c, :, :],
                             oT[h][:, c * SC:(c + 1) * SC])

        def run_a2a(h):
            if nocc:
                sdma(a2a_out[h][:], a2a_in[h][:])
            else:
                nc.gpsimd.collective_compute(
                    "AllToAll", OP.bypass,
                    replica_groups=[list(range(NCORES))],
                    ins=[a2a_in[h][:]], outs=[a2a_out[h][:]])

        attention_head(0)
        run_a2a(0)

        # q/k head1 (overlaps A2A#0), then attention head1.
        with tc.tile_pool(name="w3ps", bufs=2, space="PSUM") as pp:
            for (w_t, b_sb, dst) in ((wq_t[1], bq_sb, qT[1]),
                                     (wk_t[1], bk_sb, kT[1])):
                for qs in range(4):
                    ps = pp.tile([128, 512], f32, tag="w3qk", name="w3qk")
                    for kt in range(16):
                        nc.tensor.matmul(
                            ps[:], w_t[:, kt, :],
                            xT_sb[kt][:, qs * 512:(qs + 1) * 512],
                            start=(kt == 0), stop=(kt == 15))
                    qk_drain(dst, 1, qs, ps, b_sb)
        xT_keep.close()

        # ofT evens load (a2a#0 results) — overlaps attention head1.
        oft_keep = ExitStack()
        ofp = oft_keep.enter_context(
            tc.tile_pool(name="ofT", bufs=1, side="right"))
        ofT = [[None] * NCORES for _ in range(HPC)]
        for c in range(NCORES):
            t = ofp.tile([128, SC], bf16, tag=f"ofT0_{c}", name=f"ofT0_{c}")
            sdma(t[:], a2a_out[0][c, :, :])
            ofT[0][c] = t

        attention_head(1)
        run_a2a(1)
        att_keep.close()
        qkv_keep.close()

        for c in range(NCORES):
            t = ofp.tile([128, SC], bf16, tag=f"ofT1_{c}", name=f"ofT1_{c}")
            sdma(t[:], a2a_out[1][c, :, :])
            ofT[1][c] = t

        # Persistent pools for the FFN part + residual.  Left-stack open
        # order is chosen so releases are LIFO: res/w2/sqscr/h1T
        # (end-of-kernel) at the bottom, then up/lnA (close after
        # phase 4) on top, gi pushed after those pop.
        resA = ExitStack()
        res_pool = resA.enter_context(tc.tile_pool(name="res", bufs=1))
        w2_keep = ExitStack()
        w2p = w2_keep.enter_context(tc.tile_pool(name="w2", bufs=w2bufs))
        sqscr_keep = ExitStack()
        sq_scratch_pool = sqscr_keep.enter_context(
            tc.tile_pool(name="sqscr", bufs=2))
        h1b = [res_pool.tile([128, D], f32, tag=f"h1b{m}", name=f"h1b{m}")
               for m in range(2)]
        u_tiles = [res_pool.tile([128, D], f32, tag=f"u{m}", name=f"u{m}")
                   for m in range(2)]
        h1T_keep = ExitStack()
        h1Tp = h1T_keep.enter_context(tc.tile_pool(name="h1T", bufs=1))
        h1T = [h1Tp.tile([128, SC], bf16, tag=f"h1T{kt}", name=f"h1T{kt}")
               for kt in range(16)]
        up_keep = ExitStack()
        up = up_keep.enter_context(tc.tile_pool(name="up", bufs=1))
        z32 = [up.tile([128, D], FP, tag=f"z32_{m}", name=f"z32_{m}")
               for m in range(2)]
        lnA_keep = ExitStack()
        lnA = lnA_keep.enter_context(tc.tile_pool(name="lnA", bufs=1))

        # residual x slice (pre-biased with lin_b) straight into u tiles
        for m in range(2):
            sdma(u_tiles[m][:], xs_d[m * 128:(m + 1) * 128, :])

        def broadcast_row(pool, row_d, tag):
            # SWDGE broadcast-DMA with f32 -> bf16 cast
            t = pool.tile([128, D], bf16, tag=tag, name=tag, bufs=1)
            w2dma(t[:], row_d[None, :].to_broadcast((128, D)))
            return t

        # G/B broadcasts must hit the gpsimd ring BEFORE the ff2 stream
        # issues (w2 bufs rotation blocks the gpsimd queue afterwards).
        G1 = broadcast_row(lnA, g1_d, "G1")
        BCOMB = broadcast_row(lnA, bcomb_d, "BCOMB")

        # ff2 stream: issue on the gpsimd ring now (its last remaining
        # work) — transfers begin immediately; bufs rotation paces
        # against FFN2 consumption.  w2t[n*4+kc] covers
        # ff2[kc*2048:(kc+1)*2048, n*512:(n+1)*512].
        w2t = []
        for j in range(16):
            n, kc = j // 4, j % 4
            t = w2p.tile([128, 16, 512], bf16, tag="w2", name=f"w2t{j}")
            w2dma(t[:],
                  ff2_d[kc * 2048:(kc + 1) * 2048, n * 512:(n + 1) * 512]
                  .rearrange("(k p) c -> p k c", p=128))
            w2t.append(t)

        # chunked LN state: per (m, n) partial sums
        ln1_mu = [[stat.tile([128, 1], f32, tag=f"l1mu{m}_{n}",
                             name=f"l1mu{m}_{n}", bufs=1)
                   for n in range(4)] for m in range(2)]
        ln1_sq = [[stat.tile([128, 1], f32, tag=f"l1sq{m}_{n}",
                             name=f"l1sq{m}_{n}", bufs=1)
                   for n in range(4)] for m in range(2)]
        ln2_mu = [[stat.tile([128, 1], f32, tag=f"l2mu{m}_{n}",
                             name=f"l2mu{m}_{n}", bufs=1)
                   for n in range(4)] for m in range(2)]
        ln2_sq = [[stat.tile([128, 1], f32, tag=f"l2sq{m}_{n}",
                             name=f"l2sq{m}_{n}", bufs=1)
                   for n in range(4)] for m in range(2)]

        def chunk_stats(u_tile, n, mu_t, sq_t):
            ch = u_tile[:, n * 512:(n + 1) * 512]
            nc.vector.reduce_sum(mu_t[:], ch, axis=AX.X)
            scr = sq_scratch_pool.tile([128, 512], f32, tag="sqs",
                                       name="sqs")
            nc.scalar.activation(scr[:], ch, AF.Square, bias=zc[:],
                                 accum_out=sq_t[:])

        def ln_finalize(mu_list, sq_list, tag):
            musum = stat.tile([128, 1], f32, tag=f"msum{tag}",
                              name=f"msum{tag}", bufs=2)
            nc.vector.tensor_tensor(musum[:], mu_list[0][:], mu_list[1][:],
                                    OP.add)
            nc.vector.tensor_tensor(musum[:], musum[:], mu_list[2][:],
                                    OP.add)
            nc.vector.tensor_tensor(musum[:], musum[:], mu_list[3][:],
                                    OP.add)
            sqsum = stat.tile([128, 1], f32, tag=f"ssum{tag}",
                              name=f"ssum{tag}", bufs=2)
            nc.vector.tensor_tensor(sqsum[:], sq_list[0][:], sq_list[1][:],
                                    OP.add)
            nc.vector.tensor_tensor(sqsum[:], sqsum[:], sq_list[2][:],
                                    OP.add)
            nc.vector.tensor_tensor(sqsum[:], sqsum[:], sq_list[3][:],
                                    OP.add)
            mu = stat.tile([128, 1], f32, tag=f"mu{tag}", name=f"mu{tag}",
                           bufs=2)
            nc.vector.tensor_scalar(mu[:], musum[:], 1.0 / D, None, OP.mult)
            ex2 = stat.tile([128, 1], f32, tag=f"ex2{tag}",
                            name=f"ex2{tag}", bufs=2)
            nc.vector.tensor_scalar(ex2[:], sqsum[:], 1.0 / D, None, OP.mult)
            mu2 = stat.tile([128, 1], f32, tag=f"mu2{tag}", name=f"mu2{tag}",
                            bufs=2)
            nc.vector.tensor_tensor(mu2[:], mu[:], mu[:], OP.mult)
            var = stat.tile([128, 1], f32, tag=f"var{tag}", name=f"var{tag}",
                            bufs=2)
            nc.vector.tensor_tensor(var[:], ex2[:], mu2[:], OP.subtract)
            std = stat.tile([128, 1], f32, tag=f"std{tag}", name=f"std{tag}",
                            bufs=2)
            nc.scalar.activation(std[:], var[:], AF.Sqrt, bias=eps_sb[:])
            rstd = stat.tile([128, 1], f32, tag=f"rstd{tag}",
                             name=f"rstd{tag}", bufs=2)
            nc.vector.reciprocal(rstd[:], std[:])
            return mu, rstd

        # ---------------- Phase 3: out-proj (evens overlap A2A#1) --------
        with tc.tile_pool(name="op_ps", bufs=8, space="PSUM") as opp:
            ops = [[opp.tile([128, 512], f32, tag=f"op{n}_{m}",
                             name=f"op{n}_{m}", bufs=1)
                    for m in range(2)] for n in range(4)]
            for h in range(HPC):
                for n in range(4):
                    for m in range(2):
                        for c in range(NCORES):
                            kt = 2 * c + h
                            nc.tensor.matmul(
                                ops[n][m][:],
                                ofT[h][c][:, m * 128:(m + 1) * 128],
                                lwt[n][:, kt, :],
                                start=(h == 0 and c == 0),
                                stop=(h == HPC - 1 and c == NCORES - 1))
                        if h == HPC - 1:
                            nc.vector.tensor_tensor(
                                u_tiles[m][:, n * 512:(n + 1) * 512],
                                ops[n][m][:],
                                u_tiles[m][:, n * 512:(n + 1) * 512],
                                OP.add)
                            chunk_stats(u_tiles[m], n, ln1_mu[m][n],
                                        ln1_sq[m][n])
        oft_keep.close()

        # ff1 stream on the sync ring: tiles rotate through the wbig
        # slots, so each transfer begins as soon as the matching lin_w
        # slot's last out-proj read is done.
        w1t = []
        for ib in range(16):
            t = wbp.tile([128, 16, 512], bf16, tag="wbig", name=f"w1t{ib}")
            sdma(t[:],
                 ff1_d[:, ib * 512:(ib + 1) * 512]
                 .rearrange("(k p) c -> p k c", p=128))
            w1t.append(t)

        # ---------------- Phase 4: LN1 finalize + z + transpose ----------
        with tc.tile_pool(name="tr_ps", bufs=3, space="PSUM") as tpp:
            mr = [ln_finalize(ln1_mu[m], ln1_sq[m], f"1_{m}")
                  for m in range(2)]
            for n in range(4):
                for m in range(2):
                    nc.vector.tensor_scalar(
                        z32[m][:, n * 512:(n + 1) * 512],
                        u_tiles[m][:, n * 512:(n + 1) * 512],
                        mr[m][0][:], mr[m][1][:], OP.subtract, OP.mult)
                for kt in range(4 * n, 4 * n + 4):
                    for m in range(2):
                        tp = tpp.tile([128, 128], FP, tag="tr_ps",
                                      name="tr_ps")
                        nc.tensor.transpose(
                            tp[:], z32[m][:, kt * 128:(kt + 1) * 128],
                            ident[:])
                        nc.vector.tensor_copy(
                            h1T[kt][:, m * 128:(m + 1) * 128], tp[:])
            # residual h1b = z32*G1 + BCOMB (lazy, off the critical path)
            for m in range(2):
                nc.vector.tensor_tensor(h1b[m][:], z32[m][:], G1[:],
                                        OP.mult)
                nc.vector.tensor_tensor(h1b[m][:], h1b[m][:], BCOMB[:],
                                        OP.add)
        lnA_keep.close()
        up_keep.close()

        # ---------------- Phase 5: FFN1 (seq-parallel) -------------------
        gi_keep = ExitStack()
        gip = gi_keep.enter_context(tc.tile_pool(name="gi", bufs=1))
        ginner = []
        with tc.tile_pool(name="ffn_ps", bufs=3, space="PSUM") as fpp:
            for ib in range(16):
                w1tt = w1t[ib]
                for ms in range(4):
                    it = ib * 4 + ms
                    ps = fpp.tile([128, SC], f32, tag="f1_ps", name="f1_ps")
                    for kt in range(16):
                        nc.tensor.matmul(
                            ps[:], w1tt[:, kt, ms * 128:(ms + 1) * 128],
                            h1T[kt][:], start=(kt == 0), stop=(kt == 15))
                    g = gip.tile([128, SC], bf16, tag=f"gi{it}",
                                 name=f"gi{it}")
                    nc.scalar.activation(g[:], ps[:], AF.Gelu,
                                         bias=ff1b_sb[:, it:it + 1])
                    ginner.append(g)
        wbig_keep.close()

        # -------- Phase 6: FFN2 + LN2 chunked stats ----------------------
        with tc.tile_pool(name="f2_ps", bufs=4, space="PSUM") as f2p:
            for n in range(4):
                pss = [f2p.tile([128, 512], f32, tag=f"f2_ps{m}",
                                name=f"f2ps{m}") for m in range(2)]
                for kc in range(4):
                    w2tt = w2t[n * 4 + kc]
                    for m in range(2):
                        for k2 in range(16):
                            kt = kc * 16 + k2
                            nc.tensor.matmul(
                                pss[m][:],
                                ginner[kt][:, m * 128:(m + 1) * 128],
                                w2tt[:, k2, :],
                                start=(kt == 0), stop=(kt == 63))
                for m in range(2):
                    nc.vector.tensor_tensor(
                        u_tiles[m][:, n * 512:(n + 1) * 512], pss[m][:],
                        h1b[m][:, n * 512:(n + 1) * 512], OP.add)
                    chunk_stats(u_tiles[m], n, ln2_mu[m][n], ln2_sq[m][n])

            # -------- Phase 7: LN2 normalize (z2) + store ----------------
            for m in range(2):
                mu, rstd = ln_finalize(ln2_mu[m], ln2_sq[m], f"2_{m}")
                for n in range(4):
                    ch = u_tiles[m][:, n * 512:(n + 1) * 512]
                    nc.vector.tensor_scalar(ch, ch, mu[:], rstd[:],
                                            OP.subtract, OP.mult)
                    sdma(out_d[m * 128:(m + 1) * 128,
                               n * 512:(n + 1) * 512], ch)
        gi_keep.close()
        h1T_keep.close()
        sqscr_keep.close()
        w2_keep.close()
        resA.close()

    nc.compile()
    return nc


_NC_CACHE = {}


def _get_nc(debug=False, nocc=False, **kw):
    key = (nocc, tuple(sorted(kw.items())))
    if key not in _NC_CACHE:
        _NC_CACHE[key] = _build(nocc, **kw)
    return _NC_CACHE[key]


def make_in_maps(x, C_w, C_b, lin_w, lin_b, ff1_w, ff1_b, ff2_w, ff2_b,
                 ln1_g, ln1_b, ln2_g, ln2_b):
    x2 = np.asarray(x, dtype=np.float32)[0]            # [S, D]
    xT = np.ascontiguousarray(x2.T).astype(ml_dtypes.bfloat16)
    C_w = np.asarray(C_w, dtype=np.float32)
    C_b = np.asarray(C_b, dtype=np.float32)
    lin_b64 = np.asarray(lin_b, dtype=np.float64)
    ff1_w64 = np.asarray(ff1_w, dtype=np.float64)
    ln1_g64 = np.asarray(ln1_g, dtype=np.float64)
    ln1_b64 = np.asarray(ln1_b, dtype=np.float64)
    ff1w_scaled = (ff1_w64 * ln1_g64[:, None]).astype(np.float32)
    ff1b_eff = (np.asarray(ff1_b, np.float64)
                + ln1_b64 @ ff1_w64).astype(np.float32)
    bcomb = (ln1_b64 + np.asarray(ff2_b, np.float64)).astype(np.float32)
    common = {
        "xT": xT,
        "lin_w": np.asarray(lin_w).astype(ml_dtypes.bfloat16),
        "ff1_w": ff1w_scaled.astype(ml_dtypes.bfloat16),
        "ff1_b": ff1b_eff,
        "ff2_w": np.asarray(ff2_w).astype(ml_dtypes.bfloat16),
        "g1": np.asarray(ln1_g, dtype=np.float32),
        "bcomb": bcomb,
    }
    in_maps = []
    for c in range(NCORES):
        sl = slice(c * HDC, (c + 1) * HDC)
        m = dict(common)
        m["wq"] = np.ascontiguousarray(C_w[:, sl]).astype(ml_dtypes.bfloat16)
        m["wk"] = np.ascontiguousarray(
            C_w[:, D:][:, sl]).astype(ml_dtypes.bfloat16)
        m["wv"] = np.ascontiguousarray(
            C_w[:, 2 * D:][:, sl]).astype(ml_dtypes.bfloat16)
        m["bq"] = np.ascontiguousarray(C_b[sl])
        m["bk"] = np.ascontiguousarray(C_b[D:][sl])
        m["bv"] = np.ascontiguousarray(C_b[2 * D:][sl])
        m["x_slice"] = (x2[c * SC:(c + 1) * SC, :].astype(np.float64)
                        + lin_b64[None, :]).astype(np.float32)
        in_maps.append(m)
    return in_maps


def run(in_maps, debug=False):
    nc = _get_nc(debug)
    return run_bass_kernel_spmd(nc, in_maps, list(range(NCORES)))


def kernel(**inputs):
    in_maps = make_in_maps(**inputs)
    res = run(in_maps)
    z2 = np.concatenate(
        [res.results[c]["out_slice"] for c in range(NCORES)], axis=0)
    g2 = np.asarray(inputs["ln2_g"], dtype=np.float32)
    b2 = np.asarray(inputs["ln2_b"], dtype=np.float32)
    out = z2 * g2[None, :] + b2[None, :]
    return out.reshape(1, S, D).astype(np.float32)
